# revision 1
# baseline (speedup 1.0000x reference)
import sys

sys.path.insert(0, "/opt/trn_rl_repo")

import numpy as np

D_MODEL = 1024
NUM_HEADS = 16
HEAD_DIM = 64
B = 2
S = 2048
N_CORES = 8
HG = 4          # head-groups (cores per batch)
HPC = 4         # heads per core
DL = 256        # local feature width per core (HPC * HEAD_DIM)

_cache = {}
last_exec_time_ns = None


def _build(has_qkvb):
    import concourse.bacc as bacc
    import concourse.mybir as mybir
    import concourse.tile as tile

    F32 = mybir.dt.float32
    F32R = mybir.dt.float32r
    Exp = mybir.ActivationFunctionType.Exp
    mult = mybir.AluOpType.mult
    is_ge = mybir.AluOpType.is_ge

    nc = bacc.Bacc("TRN2", target_bir_lowering=False, debug=False)
    xT_d = nc.dram_tensor("xT", (D_MODEL, S), F32, kind="ExternalInput")
    wq_d = nc.dram_tensor("wqkvT", (D_MODEL, 3 * DL), F32, kind="ExternalInput")
    wo_d = nc.dram_tensor("woT", (DL, D_MODEL), F32, kind="ExternalInput")
    if has_qkvb:
        qb_d = nc.dram_tensor("qb", (1, 3 * DL), F32, kind="ExternalInput")
    out_d = nc.dram_tensor("out", (S, D_MODEL), F32, kind="ExternalOutput")

    def r(ap):
        return ap.bitcast(F32R)

    with tile.TileContext(nc) as tc:
        with tc.tile_pool(name="persist", bufs=1) as persist:
            # Q/K packed per head-pair p: partitions 0:64 head 2p, 64:128 head 2p+1
            QT = [persist.tile([128, S], F32, name=f"QT{p}") for p in range(2)]
            KT = [persist.tile([128, S], F32, name=f"KT{p}") for p in range(2)]
            # V augmented: per s-tile block of 128 cols: [V dims 64 | ones 64]
            Vaug = [persist.tile([128, S], F32, name=f"Vg{h}") for h in range(HPC)]
            # prebaked causal band masks: mask[t][k, q] = 1 if q >= k + 128t else 0
            maskt = [persist.tile([128, 512], F32, name=f"mask{t}") for t in range(4)]
            for t in range(4):
                nc.vector.memset(maskt[t][:], 1.0)
                w = 128 * (t + 1)
                nc.gpsimd.affine_select(
                    out=r(maskt[t][:, 0:w]), in_=r(maskt[t][:, 0:w]),
                    pattern=[[1, w]],
                    channel_multiplier=-1,
                    base=-128 * t,
                    compare_op=is_ge,
                    fill=0.0,
                )

            with tc.tile_pool(name="work", bufs=1) as work:
                with tc.tile_pool(name="projin", bufs=1) as projin, \
                     tc.tile_pool(name="pproj", bufs=1, space="PSUM") as pproj:
                    xt = [projin.tile([128, S], F32, name=f"xt{i}") for i in range(8)]
                    wq = [projin.tile([128, 3 * DL], F32, name=f"wq{i}") for i in range(8)]
                    for i in range(8):
                        nc.sync.dma_start(out=r(xt[i][:]), in_=r(xT_d[128 * i:128 * (i + 1), :]))
                        nc.sync.dma_start(out=r(wq[i][:]), in_=r(wq_d[128 * i:128 * (i + 1), :]))
                    if has_qkvb:
                        qb_t = projin.tile([1, 3 * DL], F32, name="qb_t")
                        nc.sync.dma_start(out=r(qb_t[:]), in_=r(qb_d[:]))
                        ones_t = projin.tile([1, 512], F32, name="ones_t")
                        nc.vector.memset(ones_t[:], 1.0)

                    # ---- QK projection: mi 0/1 -> QT[0/1], 2/3 -> KT[0/1]
                    for mi in range(4):
                        dst = QT[mi] if mi < 2 else KT[mi - 2]
                        for n in range(4):
                            psq = pproj.tile([128, 512], F32, tag="qk", bufs=2, name="psq")
                            for i in range(8):
                                nc.tensor.matmul(
                                    out=psq[:],
                                    lhsT=r(wq[i][:, 128 * mi:128 * (mi + 1)]),
                                    rhs=r(xt[i][:, 512 * n:512 * (n + 1)]),
                                    start=(i == 0),
                                    stop=(i == 7 and not has_qkvb),
                                )
                            if has_qkvb:
                                nc.tensor.matmul(
                                    out=psq[:],
                                    lhsT=r(qb_t[0:1, 128 * mi:128 * (mi + 1)]),
                                    rhs=r(ones_t[0:1, :]),
                                    start=False, stop=True,
                                )
                            nc.vector.tensor_copy(out=r(dst[:, 512 * n:512 * (n + 1)]), in_=psq[:])

                    # ---- V projection into Vaug (interleaved [V|ones] blocks)
                    for h in range(HPC):
                        nc.vector.memset(Vaug[h][:], 1.0)
                    for st in range(16):
                        psv = pproj.tile([128, DL], F32, tag="v", bufs=2, name="psv")
                        for i in range(8):
                            nc.tensor.matmul(
                                out=psv[:],
                                lhsT=r(xt[i][:, 128 * st:128 * (st + 1)]),
                                rhs=r(wq[i][:, 512:768]),
                                start=(i == 0),
                                stop=(i == 7 and not has_qkvb),
                            )
                        if has_qkvb:
                            nc.tensor.matmul(
                                out=psv[:],
                                lhsT=r(ones_t[0:1, 0:128]),
                                rhs=r(qb_t[0:1, 512:768]),
                                start=False, stop=True,
                            )
                        for h in range(HPC):
                            nc.vector.tensor_copy(
                                out=r(Vaug[h][:, 128 * st:128 * st + 64]),
                                in_=psv[:, 64 * h:64 * h + 64],
                            )

                with tc.tile_pool(name="persist2", bufs=1) as persist2:
                    # ctx pair-packed: head 2p at partitions 0:64, head 2p+1 at 64:128
                    ctxp = [persist2.tile([128, S], F32, name=f"ctxp{p}") for p in range(2)]
                    wop = [persist2.tile([128, D_MODEL], F32, name=f"wop{p}") for p in range(2)]
                    for p in range(2):
                        nc.sync.dma_start(out=r(wop[p][:]), in_=r(wo_d[128 * p:128 * (p + 1), :]))

                    # ---- attention
                    with tc.tile_pool(name="pattn", bufs=1, space="PSUM") as pattn:
                        def issue_scores(p, j, m):
                            psS0 = pattn.tile([128, 512], F32, tag="s0", bufs=2, name="psS0")
                            psS1 = pattn.tile([128, 512], F32, tag="s1", bufs=2, name="psS1")
                            nc.tensor.matmul(
                                out=psS0[:],
                                lhsT=r(KT[p][0:64, 128 * m:128 * (m + 1)]),
                                rhs=r(QT[p][0:64, 512 * j:512 * (j + 1)]),
                                start=True, stop=True,
                            )
                            nc.tensor.matmul(
                                out=psS1[:],
                                lhsT=r(KT[p][64:128, 128 * m:128 * (m + 1)]),
                                rhs=r(QT[p][64:128, 512 * j:512 * (j + 1)]),
                                start=True, stop=True,
                            )
                            return psS0, psS1

                        pjs = [(p, j) for p in range(2) for j in range(4)]
                        pending = issue_scores(*pjs[0], 0)
                        for pi, (p, j) in enumerate(pjs):
                            mlast = 4 * j + 3
                            psA = pattn.tile([128, 512], F32, tag="a", bufs=2, name="psA")
                            psB = pattn.tile([128, 512], F32, tag="b", bufs=2, name="psB")
                            for m in range(4 * j + 4):
                                psS0, psS1 = pending
                                if m < mlast:
                                    pending = issue_scores(p, j, m + 1)
                                elif pi + 1 < len(pjs):
                                    pending = issue_scores(*pjs[pi + 1], 0)
                                e0 = work.tile([128, 512], F32, tag="e0", bufs=3, name="e0")
                                e1 = work.tile([128, 512], F32, tag="e1", bufs=3, name="e1")
                                t = m - 4 * j
                                if t >= 0:
                                    # band tile: cols < 128t are fully below the causal
                                    # diagonal -> zero; exp only live cols, mask only the
                                    # 128-col partial band
                                    w0 = 128 * t
                                    nc.scalar.activation(
                                        r(e0[:, w0:512]), psS0[:, w0:512], Exp, scale=0.125)
                                    nc.scalar.activation(
                                        r(e1[:, w0:512]), psS1[:, w0:512], Exp, scale=0.125)
                                    nc.vector.tensor_tensor(
                                        out=r(e0[:, w0:w0 + 128]), in0=e0[:, w0:w0 + 128],
                                        in1=maskt[t][:, w0:w0 + 128], op=mult)
                                    nc.vector.tensor_tensor(
                                        out=r(e1[:, w0:w0 + 128]), in0=e1[:, w0:w0 + 128],
                                        in1=maskt[t][:, w0:w0 + 128], op=mult)
                                else:
                                    nc.scalar.activation(r(e0[:]), psS0[:], Exp, scale=0.125)
                                    nc.scalar.activation(r(e1[:]), psS1[:], Exp, scale=0.125)
                                lo = 128 * t if t > 0 else 0
                                nc.tensor.matmul(
                                    out=psA[:, lo:512],
                                    lhsT=r(Vaug[2 * p][:, 128 * m:128 * (m + 1)]),
                                    rhs=r(e0[:, lo:512]),
                                    start=(m == 0), stop=(m == mlast),
                                )
                                nc.tensor.matmul(
                                    out=psB[:, lo:512],
                                    lhsT=r(Vaug[2 * p + 1][:, 128 * m:128 * (m + 1)]),
                                    rhs=r(e1[:, lo:512]),
                                    start=(m == 0), stop=(m == mlast),
                                )
                            # normalize: ctxp[p][0:64|64:128, j] = psX[0:64] / sums
                            sums = work.tile([64, 512], F32, tag="sums", bufs=2, name="sums")
                            nc.vector.tensor_copy(out=sums[:], in_=psA[64:128, :])
                            rec = work.tile([64, 512], F32, tag="rec", bufs=2, name="rec")
                            nc.vector.reciprocal_approx_fast(rec[:], sums[:])
                            nc.vector.tensor_tensor(
                                out=r(ctxp[p][0:64, 512 * j:512 * (j + 1)]),
                                in0=psA[0:64, :],
                                in1=rec[:],
                                op=mult,
                            )
                            sums2 = work.tile([64, 512], F32, tag="sums", bufs=2, name="sums")
                            nc.vector.tensor_copy(out=sums2[:], in_=psB[64:128, :])
                            rec2 = work.tile([64, 512], F32, tag="rec", bufs=2, name="rec")
                            nc.vector.reciprocal_approx_fast(rec2[:], sums2[:])
                            codd = work.tile([64, 512], F32, tag="codd", bufs=2, name="codd")
                            nc.vector.tensor_tensor(
                                out=codd[:], in0=psB[0:64, :], in1=rec2[:], op=mult)
                            nc.vector.tensor_copy(
                                out=r(ctxp[p][64:128, 512 * j:512 * (j + 1)]), in_=codd[:])

                    # ---- output projection
                    with tc.tile_pool(name="outst", bufs=1) as outst, \
                         tc.tile_pool(name="pout", bufs=1, space="PSUM") as pout:
                        for qm in range(16):
                            stage = outst.tile([128, D_MODEL], F32, tag="st", bufs=3, name="stage")
                            for n in range(2):
                                pso = pout.tile([128, 512], F32, tag=f"o{n}", bufs=2, name="pso")
                                for p in range(2):
                                    nc.tensor.matmul(
                                        out=pso[:],
                                        lhsT=r(ctxp[p][:, 128 * qm:128 * (qm + 1)]),
                                        rhs=r(wop[p][:, 512 * n:512 * (n + 1)]),
                                        start=(p == 0), stop=(p == 1),
                                    )
                                nc.vector.tensor_copy(out=stage[:, 512 * n:512 * (n + 1)], in_=pso[:])
                            nc.sync.dma_start(out=out_d[128 * qm:128 * (qm + 1), :], in_=stage[:])

    nc.finalize()
    return nc


def kernel(x, qkv_w, qkv_b, out_w, out_b):
    from concourse import bass_utils
    global last_exec_time_ns

    x = np.ascontiguousarray(np.asarray(x, dtype=np.float32))
    qkv_w = np.asarray(qkv_w, dtype=np.float32)
    qkv_b = np.asarray(qkv_b, dtype=np.float32)
    out_w = np.asarray(out_w, dtype=np.float32)
    out_b = np.asarray(out_b, dtype=np.float32)

    has_qkvb = bool(np.any(qkv_b))
    if has_qkvb not in _cache:
        _cache[has_qkvb] = _build(has_qkvb)
    nc = _cache[has_qkvb]

    in_maps = []
    for c in range(N_CORES):
        b, hg = divmod(c, HG)
        xT = np.ascontiguousarray(x[b].T)
        rows = np.concatenate([
            qkv_w[DL * hg:DL * (hg + 1)],
            qkv_w[D_MODEL + DL * hg:D_MODEL + DL * (hg + 1)],
            qkv_w[2 * D_MODEL + DL * hg:2 * D_MODEL + DL * (hg + 1)],
        ], axis=0)
        wqkvT = np.ascontiguousarray(rows.T)
        woT = np.ascontiguousarray(out_w[:, DL * hg:DL * (hg + 1)].T)
        m = {"xT": xT, "wqkvT": wqkvT, "woT": woT}
        if has_qkvb:
            m["qb"] = np.concatenate([
                qkv_b[DL * hg:DL * (hg + 1)],
                qkv_b[D_MODEL + DL * hg:D_MODEL + DL * (hg + 1)],
                qkv_b[2 * D_MODEL + DL * hg:2 * D_MODEL + DL * (hg + 1)],
            ]).reshape(1, 3 * DL).astype(np.float32)
        in_maps.append(m)

    res = bass_utils.run_bass_kernel_spmd(nc, in_maps, core_ids=list(range(N_CORES)))
    last_exec_time_ns = res.exec_time_ns

    out = np.zeros((B, S, D_MODEL), dtype=np.float32)
    for c in range(N_CORES):
        b, hg = divmod(c, HG)
        out[b] += res.results[c]["out"]
    out += out_b[None, None, :]
    return out



# revision 3
# speedup vs baseline: 1.9644x; 1.9644x over previous
import sys

sys.path.insert(0, "/opt/trn_rl_repo")

import numpy as np

D_MODEL = 1024
NUM_HEADS = 16
HEAD_DIM = 64
B = 2
S = 2048
N_CORES = 8
HG = 4          # head-groups (cores per batch)
HPC = 4         # heads per core
DL = 256        # local feature width per core (HPC * HEAD_DIM)

_cache = {}
last_exec_time_ns = None

DRIP = 4        # filler instructions interleaved per attention m-step


def _build(has_qkvb):
    import concourse.bacc as bacc
    import concourse.mybir as mybir
    import concourse.tile as tile

    F32 = mybir.dt.float32
    F32R = mybir.dt.float32r
    BF16 = mybir.dt.bfloat16
    Exp = mybir.ActivationFunctionType.Exp
    mult = mybir.AluOpType.mult
    is_ge = mybir.AluOpType.is_ge

    nc = bacc.Bacc("TRN2", target_bir_lowering=False, debug=False)
    xT_d = nc.dram_tensor("xT", (D_MODEL, S), BF16, kind="ExternalInput")
    wq_d = nc.dram_tensor("wqkvT", (D_MODEL, 3 * DL), BF16, kind="ExternalInput")
    wo_d = nc.dram_tensor("woT", (DL, D_MODEL), BF16, kind="ExternalInput")
    if has_qkvb:
        qb_d = nc.dram_tensor("qb", (1, 3 * DL), BF16, kind="ExternalInput")
    out_d = nc.dram_tensor("out", (S, D_MODEL), F32, kind="ExternalOutput")

    with tile.TileContext(nc) as tc:
        with tc.tile_pool(name="persist", bufs=1) as persist, \
             tc.tile_pool(name="work", bufs=1) as work, \
             tc.tile_pool(name="pmm", bufs=1, space="PSUM") as pmm:

            xt = [persist.tile([128, S], BF16, name=f"xt{i}") for i in range(8)]
            wq = [persist.tile([128, 3 * DL], BF16, name=f"wq{i}") for i in range(8)]
            wo2 = [persist.tile([128, D_MODEL], BF16, name=f"wo{p}") for p in range(2)]
            # Q/K packed per head-pair p: partitions 0:64 head 2p, 64:128 head 2p+1
            QT = [persist.tile([128, S], BF16, name=f"QT{p}") for p in range(2)]
            KT = [persist.tile([128, S], BF16, name=f"KT{p}") for p in range(2)]
            # V augmented [pair, parity, S]: per key-tile block of 128 cols:
            # [V dims 64 | ones 64]; partitions of block st = keys of tile st
            Vaug = persist.tile([128, 2, 2, S], BF16, name="Vaug")
            ctxp = [persist.tile([128, S], BF16, name=f"ctxp{p}") for p in range(2)]
            maskf = persist.tile([128, 128], F32, name="maskf")
            maskb = persist.tile([128, 128], BF16, name="maskb")

            # DMAs in need order: weights first, then x column-chunks
            for i in range(8):
                nc.sync.dma_start(out=wq[i][:], in_=wq_d[128 * i:128 * (i + 1), :])
            for i in range(8):
                nc.sync.dma_start(out=xt[i][:, 0:512], in_=xT_d[128 * i:128 * (i + 1), 0:512])
            if has_qkvb:
                qb_t = persist.tile([1, 3 * DL], BF16, name="qb_t")
                nc.sync.dma_start(out=qb_t[:], in_=qb_d[:])
                ones_t = persist.tile([1, 512], BF16, name="ones_t")
                nc.vector.memset(ones_t[:], 1.0)
            for p in range(2):
                nc.sync.dma_start(out=wo2[p][:], in_=wo_d[128 * p:128 * (p + 1), :])
            for n in range(1, 4):
                for i in range(8):
                    nc.sync.dma_start(
                        out=xt[i][:, 512 * n:512 * (n + 1)],
                        in_=xT_d[128 * i:128 * (i + 1), 512 * n:512 * (n + 1)])

            # causal diag-block mask: maskb[k, q] = 1 if q >= k (within 128x128 tile)
            nc.vector.memset(maskf[:], 1.0)
            nc.gpsimd.affine_select(
                out=maskf[:].bitcast(F32R), in_=maskf[:].bitcast(F32R),
                pattern=[[1, 128]],
                channel_multiplier=-1,
                base=0,
                compare_op=is_ge,
                fill=0.0,
            )
            nc.vector.tensor_copy(out=maskb[:], in_=maskf[:])
            nc.vector.memset(Vaug[:], 1.0)

            # ---- projection / out-projection instruction generators ----
            def qk_items(mi, n):
                # psq = sum_i wq[i][:, mi-block].T @ xt[i][:, n-chunk]  -> [128 feat, 512 seq]
                items = []
                st = {}

                def mk(i):
                    def f():
                        if i == 0:
                            st['ps'] = pmm.tile([128, 512], F32, tag="pp", bufs=2, name="psq")
                        nc.tensor.matmul(
                            out=st['ps'][:],
                            lhsT=wq[i][:, 128 * mi:128 * (mi + 1)],
                            rhs=xt[i][:, 512 * n:512 * (n + 1)],
                            start=(i == 0),
                            stop=(i == 7 and not has_qkvb),
                        )
                    return f
                for i in range(8):
                    items.append(mk(i))
                if has_qkvb:
                    def fb():
                        nc.tensor.matmul(
                            out=st['ps'][:],
                            lhsT=qb_t[0:1, 128 * mi:128 * (mi + 1)],
                            rhs=ones_t[0:1, :],
                            start=False, stop=True,
                        )
                    items.append(fb)

                def cp():
                    dst = QT[mi] if mi < 2 else KT[mi - 2]
                    nc.vector.tensor_copy(
                        out=dst[:, 512 * n:512 * (n + 1)], in_=st['ps'][:])
                items.append(cp)
                return items

            def v_items(sti):
                # psv = sum_i xt[i][:, st-block].T @ wq[i][:, V cols] -> [128 seq, 256 feat]
                items = []
                st = {}

                def mk(i):
                    def f():
                        if i == 0:
                            st['ps'] = pmm.tile([128, 2, 128], F32, tag="pp", bufs=2, name="psv")
                        nc.tensor.matmul(
                            out=st['ps'][:],
                            lhsT=xt[i][:, 128 * sti:128 * (sti + 1)],
                            rhs=wq[i][:, 512:768],
                            start=(i == 0),
                            stop=(i == 7 and not has_qkvb),
                        )
                    return f
                for i in range(8):
                    items.append(mk(i))
                if has_qkvb:
                    def fb():
                        nc.tensor.matmul(
                            out=st['ps'][:],
                            lhsT=ones_t[0:1, 0:128],
                            rhs=qb_t[0:1, 512:768],
                            start=False, stop=True,
                        )
                    items.append(fb)

                def cpe():
                    nc.vector.tensor_copy(
                        out=Vaug[:, :, 0, 128 * sti:128 * sti + 64],
                        in_=st['ps'][:, :, 0:64])

                def cpo():
                    nc.vector.tensor_copy(
                        out=Vaug[:, :, 1, 128 * sti:128 * sti + 64],
                        in_=st['ps'][:, :, 64:128])
                items.append(cpe)
                items.append(cpo)
                return items

            def outproj_items(qm):
                items = []
                st = {}

                def half(n):
                    def f():
                        ps = pmm.tile([128, 512], F32, tag="pp", bufs=2, name="pso")
                        nc.tensor.matmul(
                            out=ps[:],
                            lhsT=ctxp[0][:, 128 * qm:128 * (qm + 1)],
                            rhs=wo2[0][:, 512 * n:512 * (n + 1)],
                            start=True, stop=False,
                        )
                        nc.tensor.matmul(
                            out=ps[:],
                            lhsT=ctxp[1][:, 128 * qm:128 * (qm + 1)],
                            rhs=wo2[1][:, 512 * n:512 * (n + 1)],
                            start=False, stop=True,
                        )
                        if n == 0:
                            st['stage'] = work.tile([128, D_MODEL], F32, tag="st", bufs=3, name="stage")
                        nc.vector.tensor_copy(
                            out=st['stage'][:, 512 * n:512 * (n + 1)], in_=ps[:])
                    return f
                items.append(half(0))
                items.append(half(1))

                def dm():
                    nc.sync.dma_start(out=out_d[128 * qm:128 * (qm + 1), :], in_=st['stage'][:])
                items.append(dm)
                return items

            # ---- immediate emission: minimum needed for attn(0, 0) ----
            for it in qk_items(0, 0) + qk_items(2, 0):
                it()
            for sti in range(4):
                for it in v_items(sti):
                    it()

            # ---- filler queue for the rest, drained during attention ----
            FQ = []
            need_idx = {(0, 0): 0}
            for n in range(1, 4):
                FQ += qk_items(0, n) + qk_items(2, n)
                for sti in range(4 * n, 4 * n + 4):
                    FQ += v_items(sti)
                need_idx[(0, n)] = len(FQ)
            FQ += qk_items(1, 0) + qk_items(3, 0)
            need_idx[(1, 0)] = len(FQ)
            for n in range(1, 4):
                FQ += qk_items(1, n) + qk_items(3, n)
                need_idx[(1, n)] = len(FQ)

            drained = [0]

            def drain_to(k):
                while drained[0] < k:
                    FQ[drained[0]]()
                    drained[0] += 1

            def drip(r):
                drain_to(min(drained[0] + r, len(FQ)))

            # ---- attention ----
            def issue_scores(p, j, m):
                t = m - 4 * j
                lo = 128 * t if t > 0 else 0
                ps = pmm.tile([128, 2, 512], F32, tag="s", bufs=2, name="psS")
                nc.tensor.matmul(
                    out=ps[:, 0, lo:512],
                    lhsT=KT[p][0:64, 128 * m:128 * (m + 1)],
                    rhs=QT[p][0:64, 512 * j + lo:512 * (j + 1)],
                    start=True, stop=True,
                )
                nc.tensor.matmul(
                    out=ps[:, 1, lo:512],
                    lhsT=KT[p][64:128, 128 * m:128 * (m + 1)],
                    rhs=QT[p][64:128, 512 * j + lo:512 * (j + 1)],
                    start=True, stop=True,
                )
                return ps

            blocks = [(p, j) for p in range(2) for j in range(4)]
            psprev = issue_scores(0, 0, 0)
            for bi, (p, j) in enumerate(blocks):
                drain_to(need_idx[(p, j)])
                mlast = 4 * j + 3
                psA = pmm.tile([128, 512], F32, tag="a", bufs=1, name="psA")
                psB = pmm.tile([128, 512], F32, tag="b", bufs=1, name="psB")
                for m in range(mlast + 1):
                    ps = psprev
                    if m < mlast:
                        psprev = issue_scores(p, j, m + 1)
                    elif bi + 1 < len(blocks):
                        psprev = issue_scores(*blocks[bi + 1], 0)
                    t = m - 4 * j
                    w0 = 128 * t if t > 0 else 0
                    e = work.tile([128, 2, 512], BF16, tag="e", bufs=3, name="e")
                    nc.scalar.activation(
                        e[:, :, w0:512], ps[:, :, w0:512], Exp, scale=0.125)
                    if t >= 0:
                        # partial 128-col diagonal band: zero q < k entries
                        nc.vector.tensor_tensor(
                            out=e[:, 0, w0:w0 + 128], in0=e[:, 0, w0:w0 + 128],
                            in1=maskb[:], op=mult)
                        nc.vector.tensor_tensor(
                            out=e[:, 1, w0:w0 + 128], in0=e[:, 1, w0:w0 + 128],
                            in1=maskb[:], op=mult)
                    nc.tensor.matmul(
                        out=psA[:, w0:512],
                        lhsT=Vaug[:, p, 0, 128 * m:128 * (m + 1)],
                        rhs=e[:, 0, w0:512],
                        start=(m == 0), stop=(m == mlast),
                    )
                    nc.tensor.matmul(
                        out=psB[:, w0:512],
                        lhsT=Vaug[:, p, 1, 128 * m:128 * (m + 1)],
                        rhs=e[:, 1, w0:512],
                        start=(m == 0), stop=(m == mlast),
                    )
                    drip(DRIP)
                # normalize: ctxp[p][0:64|64:128, j-cols] = psX[0:64] / sums
                sumsE = work.tile([64, 512], F32, tag="sE", bufs=2, name="sumsE")
                nc.vector.tensor_copy(out=sumsE[:], in_=psA[64:128, :])
                recE = work.tile([64, 512], F32, tag="rE", bufs=2, name="recE")
                nc.vector.reciprocal_approx_fast(recE[:], sumsE[:])
                nc.vector.tensor_tensor(
                    out=ctxp[p][0:64, 512 * j:512 * (j + 1)],
                    in0=psA[0:64, :], in1=recE[:], op=mult)
                sumsO = work.tile([64, 512], F32, tag="sO", bufs=2, name="sumsO")
                nc.vector.tensor_copy(out=sumsO[:], in_=psB[64:128, :])
                recO = work.tile([64, 512], F32, tag="rO", bufs=2, name="recO")
                nc.vector.reciprocal_approx_fast(recO[:], sumsO[:])
                codd = work.tile([64, 512], BF16, tag="cO", bufs=2, name="codd")
                nc.vector.tensor_tensor(
                    out=codd[:], in0=psB[0:64, :], in1=recO[:], op=mult)
                nc.vector.tensor_copy(
                    out=ctxp[p][64:128, 512 * j:512 * (j + 1)], in_=codd[:])
                if p == 1:
                    for qm in range(4 * j, 4 * j + 4):
                        FQ += outproj_items(qm)
            drain_to(len(FQ))

    nc.finalize()
    return nc


def kernel(x, qkv_w, qkv_b, out_w, out_b):
    from concourse import bass_utils
    from ml_dtypes import bfloat16
    global last_exec_time_ns

    x = np.ascontiguousarray(np.asarray(x, dtype=np.float32))
    qkv_w = np.asarray(qkv_w, dtype=np.float32)
    qkv_b = np.asarray(qkv_b, dtype=np.float32)
    out_w = np.asarray(out_w, dtype=np.float32)
    out_b = np.asarray(out_b, dtype=np.float32)

    has_qkvb = bool(np.any(qkv_b))
    if has_qkvb not in _cache:
        _cache[has_qkvb] = _build(has_qkvb)
    nc = _cache[has_qkvb]

    in_maps = []
    for c in range(N_CORES):
        b, hg = divmod(c, HG)
        xT = np.ascontiguousarray(x[b].T.astype(bfloat16))
        rows = np.concatenate([
            qkv_w[DL * hg:DL * (hg + 1)],
            qkv_w[D_MODEL + DL * hg:D_MODEL + DL * (hg + 1)],
            qkv_w[2 * D_MODEL + DL * hg:2 * D_MODEL + DL * (hg + 1)],
        ], axis=0)
        wqkvT = np.ascontiguousarray(rows.T.astype(bfloat16))
        woT = np.ascontiguousarray(out_w[:, DL * hg:DL * (hg + 1)].T.astype(bfloat16))
        m = {"xT": xT, "wqkvT": wqkvT, "woT": woT}
        if has_qkvb:
            m["qb"] = np.concatenate([
                qkv_b[DL * hg:DL * (hg + 1)],
                qkv_b[D_MODEL + DL * hg:D_MODEL + DL * (hg + 1)],
                qkv_b[2 * D_MODEL + DL * hg:2 * D_MODEL + DL * (hg + 1)],
            ]).reshape(1, 3 * DL).astype(bfloat16)
        in_maps.append(m)

    res = bass_utils.run_bass_kernel_spmd(nc, in_maps, core_ids=list(range(N_CORES)))
    last_exec_time_ns = res.exec_time_ns

    out = np.zeros((B, S, D_MODEL), dtype=np.float32)
    for c in range(N_CORES):
        b, hg = divmod(c, HG)
        out[b] += res.results[c]["out"]
    out += out_b[None, None, :]
    return out
